# revision 1
# baseline (speedup 1.0000x reference)
"""CoAttention kernel for 8 Trainium2 NeuronCores.

Math (per batch b), algebraically refactored so the [Lt, Lv] affinity matrix
is never materialized:
    wq_q = T @ w_q                    [Lt, K]
    wv_v = I @ w_v                    [Lv, K]
    A1   = T^T @ wq_q                 [E, K]
    B1   = I^T @ wv_v                 [E, K]
    A2   = w_b^T @ A1                 [E, K]
    B2   = w_b @ B1                   [E, K]
    wqqc = I @ A2                     [Lv, K]   (== affinity^T @ wq_q)
    wvvc = T @ B2                     [Lt, K]   (== affinity @ wv_v)
    h_v  = tanh(wv_v + wqqc); h_q = tanh(wq_q + wvvc)
    av   = softmax(h_v @ w_hv); aq = softmax(h_q @ w_hq)
    out  = tanh((av @ I + aq @ T) @ w_s)       [E]

Sharding: data-parallel over batch. B=64 -> 8 batches per core, weights
replicated. No collectives.

Precision: fp16 operands on the PE (10-bit mantissa, ~tf32-grade), fp32 PSUM
accumulation, softmax and final tanh in fp32.

The batch loop is software-pipelined: batch b's tail (logit matmuls, softmax,
context accumulation) is emitted after batch b+1's head, so the PE always has
dense matmul work while the tail's DVE/ACT dependencies resolve (keeps the
HAM clock gate at 2.4 GHz).
"""

import numpy as np

import concourse.bass as bass
import concourse.mybir as mybir
import concourse.tile as tile
from concourse import bass_utils
from concourse.masks import make_identity

# problem shape (hardcoded per contract)
B, LT, LV, E, K = 64, 1024, 576, 768, 128
N_CORES = 8
BPC = B // N_CORES  # batches per core
P = 128
EC = E // P            # 6 chunks of E
LTC = LT // P          # 8 chunks of Lt
LV_CH = [128, 128, 128, 128, 64]   # Lv = 576 = 4*128 + 64
LVC = len(LV_CH)

F32 = mybir.dt.float32
F16 = mybir.dt.float16
TANH = mybir.ActivationFunctionType.Tanh
EXP = mybir.ActivationFunctionType.Exp
COPY = mybir.ActivationFunctionType.Copy


def _split_excess_waits(nc, limit=1):
    """walrus encodes at most one sem wait per hardware instruction; hoist
    extras onto same-engine NOPs placed immediately before."""
    for f in nc.m.functions:
        for bb in f.blocks:
            new_insts = []
            for inst in bb.instructions:
                w = inst.sync_info.on_wait if inst.sync_info else None
                if w and len(w) > limit:
                    extra, keep = w[:-limit], w[-limit:]
                    for j, sw in enumerate(extra):
                        new_insts.append(
                            mybir.InstNoOp(
                                name=f"{inst.name}-waitsplit-{j}",
                                engine=inst.engine,
                                ins=[],
                                outs=[],
                                sync_info=mybir.SyncInfo(on_wait=[sw], on_update=[]),
                            )
                        )
                    inst.sync_info.on_wait = keep
                new_insts.append(inst)
            bb.instructions[:] = new_insts


def build_nc(split_drains=True):
    nc = bass.Bass("TRN2", target_bir_lowering=False, debug=False, num_devices=N_CORES)

    text = nc.dram_tensor("text", [BPC, LT, E], F32, kind="ExternalInput").ap()
    image = nc.dram_tensor("image", [BPC, LV, E], F32, kind="ExternalInput").ap()
    wq_d = nc.dram_tensor("wq", [E, K], F16, kind="ExternalInput").ap()
    wv_d = nc.dram_tensor("wv", [E, K], F16, kind="ExternalInput").ap()
    wb_d = nc.dram_tensor("wb", [E, E], F16, kind="ExternalInput").ap()
    wbT_d = nc.dram_tensor("wbT", [E, E], F16, kind="ExternalInput").ap()
    whv_d = nc.dram_tensor("whv", [K, 1], F16, kind="ExternalInput").ap()
    whq_d = nc.dram_tensor("whq", [K, 1], F16, kind="ExternalInput").ap()
    ws_d = nc.dram_tensor("ws", [E, E], F16, kind="ExternalInput").ap()
    out_d = nc.dram_tensor("out", [BPC, E], F32, kind="ExternalOutput").ap()

    with tile.TileContext(nc) as tc:
        with (
            tc.tile_pool(name="const", bufs=1) as const,
            tc.tile_pool(name="stage", bufs=4) as stage,
            tc.tile_pool(name="work", bufs=1) as work,
            tc.tile_pool(name="pst", bufs=2, space="PSUM") as pst,    # head packs
            tc.tile_pool(name="psm", bufs=3, space="PSUM") as psm,    # matmul outs
            tc.tile_pool(name="pss", bufs=2, space="PSUM") as pss,    # [1, N] outs
            tc.tile_pool(name="pstt", bufs=1, space="PSUM") as pstt,  # tail packs
        ):
            # ---- constants / weights (loaded once) ----
            id32 = const.tile([P, P], F32)
            make_identity(nc, id32)
            id16 = const.tile([P, P], F16)
            make_identity(nc, id16)

            wq_sb = const.tile([P, EC, K], F16)
            nc.sync.dma_start(wq_sb[:], wq_d.rearrange("(c p) k -> p c k", p=P))
            wv_sb = const.tile([P, EC, K], F16)
            nc.sync.dma_start(wv_sb[:], wv_d.rearrange("(c p) k -> p c k", p=P))
            wb_sb = const.tile([P, EC, E], F16)
            nc.sync.dma_start(wb_sb[:], wb_d.rearrange("(c p) e -> p c e", p=P))
            wbT_sb = const.tile([P, EC, E], F16)
            nc.sync.dma_start(wbT_sb[:], wbT_d.rearrange("(c p) e -> p c e", p=P))
            ws_sb = const.tile([P, EC, E], F16)
            nc.sync.dma_start(ws_sb[:], ws_d.rearrange("(c p) e -> p c e", p=P))
            whv_sb = const.tile([P, 1], F16)
            nc.sync.dma_start(whv_sb[:], whv_d)
            whq_sb = const.tile([P, 1], F16)
            nc.sync.dma_start(whq_sb[:], whq_d)

            # written by every batch, consumed once at the end
            Scol16 = const.tile([P, EC, BPC], F16)
            out32 = const.tile([BPC, E], F32)

            def emit_head(b):
                """loads .. tanh(h). Returns tiles the tail needs."""
                # ---- load T, I chunk-wise (fp32), convert to fp16 promptly ----
                Tn16 = work.tile([P, LTC, E], F16, tag="Tn16", bufs=2)
                In16 = work.tile([P, LVC, E], F16, tag="In16", bufs=2)
                for cx in range(LTC):
                    st = stage.tile([P, E], F32, tag="st32")
                    nc.sync.dma_start(st[:], text[b, 128 * cx : 128 * (cx + 1), :])
                    if cx < 4:
                        nc.vector.tensor_copy(Tn16[:, cx, :], st[:])
                    else:
                        nc.scalar.activation(Tn16[:, cx, :], st[:], COPY)
                for cy in range(LVC):
                    pc = LV_CH[cy]
                    st = stage.tile([P, E], F32, tag="st32")
                    nc.sync.dma_start(
                        st[0:pc, :], image[b, 128 * cy : 128 * cy + pc, :]
                    )
                    if cy < 2:
                        nc.vector.tensor_copy(In16[0:pc, cy, :], st[0:pc, :])
                    else:
                        nc.scalar.activation(In16[0:pc, cy, :], st[0:pc, :], COPY)

                # ---- transpose T -> Ttr16 [P, EC, LT] ----
                Ttr16 = work.tile([P, EC, LT], F16, tag="Ttr16")
                for ce in range(EC):
                    for h in range(2):
                        ps = pst.tile([P, 512], F16, tag="pst")
                        for j in range(4):
                            cx = 4 * h + j
                            nc.tensor.transpose(
                                ps[:, 128 * j : 128 * (j + 1)],
                                Tn16[:, cx, 128 * ce : 128 * (ce + 1)],
                                id16[:],
                            )
                        nc.vector.tensor_copy(
                            Ttr16[:, ce, 512 * h : 512 * (h + 1)], ps[:]
                        )

                # ---- transpose I -> Itr16 [P, EC, LV] ----
                Itr16 = work.tile([P, EC, LV], F16, tag="Itr16")
                for ce in range(EC):
                    ps = pst.tile([P, 512], F16, tag="pst")
                    for cy in range(4):
                        nc.tensor.transpose(
                            ps[:, 128 * cy : 128 * (cy + 1)],
                            In16[:, cy, 128 * ce : 128 * (ce + 1)],
                            id16[:],
                        )
                    nc.vector.tensor_copy(Itr16[:, ce, 0:512], ps[:])
                    ps2 = pst.tile([P, 512], F16, tag="pst")
                    nc.tensor.transpose(
                        ps2[:, 0:64],
                        In16[0:64, 4, 128 * ce : 128 * (ce + 1)],
                        id16[0:64, 0:64],
                    )
                    nc.vector.tensor_copy(Itr16[:, ce, 512:576], ps2[:, 0:64])

                # ---- S1: wq_qT [K=P, LT] = w_q^T @ T^T ----
                wqqT16 = work.tile([P, LT], F16, tag="wqqT16")
                for h in range(2):
                    ps = psm.tile([P, 512], F32, tag="psm")
                    for e in range(EC):
                        nc.tensor.matmul(
                            ps[:],
                            wq_sb[:, e, :],
                            Ttr16[:, e, 512 * h : 512 * (h + 1)],
                            start=(e == 0),
                            stop=(e == EC - 1),
                        )
                    nc.vector.tensor_copy(wqqT16[:, 512 * h : 512 * (h + 1)], ps[:])

                # ---- S2: wv_vT [K=P, LV] ----
                wvvT16 = work.tile([P, LV], F16, tag="wvvT16")
                for lo, hi in ((0, 512), (512, 576)):
                    ps = psm.tile([P, 512], F32, tag="psm")
                    for e in range(EC):
                        nc.tensor.matmul(
                            ps[:, 0 : hi - lo],
                            wv_sb[:, e, :],
                            Itr16[:, e, lo:hi],
                            start=(e == 0),
                            stop=(e == EC - 1),
                        )
                    nc.vector.tensor_copy(wvvT16[:, lo:hi], ps[:, 0 : hi - lo])

                # ---- transpose wq_qT -> wqqn16 (natural) [P, LTC, K] ----
                wqqn16 = work.tile([P, LTC, K], F16, tag="wqqn16")
                for h in range(2):
                    ps = pst.tile([P, 512], F16, tag="pst")
                    for j in range(4):
                        cx = 4 * h + j
                        nc.tensor.transpose(
                            ps[:, 128 * j : 128 * (j + 1)],
                            wqqT16[:, 128 * cx : 128 * (cx + 1)],
                            id16[:],
                        )
                    nc.vector.tensor_copy(wqqn16[:, 4 * h : 4 * (h + 1), :], ps[:])

                # ---- transpose wv_vT -> wvvn16 [P, LVC, K] ----
                wvvn16 = work.tile([P, LVC, K], F16, tag="wvvn16")
                ps = pst.tile([P, 512], F16, tag="pst")
                for cy in range(4):
                    nc.tensor.transpose(
                        ps[:, 128 * cy : 128 * (cy + 1)],
                        wvvT16[:, 128 * cy : 128 * (cy + 1)],
                        id16[:],
                    )
                nc.vector.tensor_copy(wvvn16[:, 0:4, :], ps[:])
                ps2 = pst.tile([P, 512], F16, tag="pst")
                nc.tensor.transpose(ps2[0:64, 0:128], wvvT16[:, 512:576], id16[:])
                nc.vector.tensor_copy(wvvn16[0:64, 4, :], ps2[0:64, 0:128])

                # ---- S3: A1T16 [K=P, E] = wq_q^T @ T ----
                A1T16 = work.tile([P, E], F16, tag="A1T16")
                for h in range(2):
                    ps = psm.tile([P, 512], F32, tag="psm")
                    for x in range(LTC):
                        nc.tensor.matmul(
                            ps[:, 0:384],
                            wqqn16[:, x, :],
                            Tn16[:, x, 384 * h : 384 * (h + 1)],
                            start=(x == 0),
                            stop=(x == LTC - 1),
                        )
                    nc.vector.tensor_copy(A1T16[:, 384 * h : 384 * (h + 1)], ps[:, 0:384])

                # ---- S4: B1T16 [K=P, E] = wv_v^T @ I ----
                B1T16 = work.tile([P, E], F16, tag="B1T16")
                for h in range(2):
                    ps = psm.tile([P, 512], F32, tag="psm")
                    for cy in range(LVC):
                        pc = LV_CH[cy]
                        nc.tensor.matmul(
                            ps[:, 0:384],
                            wvvn16[0:pc, cy, :],
                            In16[0:pc, cy, 384 * h : 384 * (h + 1)],
                            start=(cy == 0),
                            stop=(cy == LVC - 1),
                        )
                    nc.vector.tensor_copy(B1T16[:, 384 * h : 384 * (h + 1)], ps[:, 0:384])

                # ---- transpose A1T16/B1T16 -> natural f16 blocks ----
                A1n16 = work.tile([P, EC, K], F16, tag="A1n16")
                ps = pst.tile([P, 768], F16, tag="pst")
                for e in range(EC):
                    nc.tensor.transpose(
                        ps[:, 128 * e : 128 * (e + 1)],
                        A1T16[:, 128 * e : 128 * (e + 1)],
                        id16[:],
                    )
                nc.vector.tensor_copy(A1n16[:], ps[:, 0:E])
                B1n16 = work.tile([P, EC, K], F16, tag="B1n16")
                ps = pst.tile([P, 768], F16, tag="pst")
                for e in range(EC):
                    nc.tensor.transpose(
                        ps[:, 128 * e : 128 * (e + 1)],
                        B1T16[:, 128 * e : 128 * (e + 1)],
                        id16[:],
                    )
                nc.vector.tensor_copy(B1n16[:], ps[:, 0:E])

                # ---- S5: A2T16 [K=P, E] = A1^T @ w_b ----
                A2T16 = work.tile([P, E], F16, tag="A2T16")
                for h in range(2):
                    ps = psm.tile([P, 512], F32, tag="psm")
                    for e in range(EC):
                        nc.tensor.matmul(
                            ps[:, 0:384],
                            A1n16[:, e, :],
                            wb_sb[:, e, 384 * h : 384 * (h + 1)],
                            start=(e == 0),
                            stop=(e == EC - 1),
                        )
                    nc.vector.tensor_copy(A2T16[:, 384 * h : 384 * (h + 1)], ps[:, 0:384])

                # ---- S6: B2T16 = B1^T @ w_b^T ----
                B2T16 = work.tile([P, E], F16, tag="B2T16")
                for h in range(2):
                    ps = psm.tile([P, 512], F32, tag="psm")
                    for e in range(EC):
                        nc.tensor.matmul(
                            ps[:, 0:384],
                            B1n16[:, e, :],
                            wbT_sb[:, e, 384 * h : 384 * (h + 1)],
                            start=(e == 0),
                            stop=(e == EC - 1),
                        )
                    nc.vector.tensor_copy(B2T16[:, 384 * h : 384 * (h + 1)], ps[:, 0:384])

                # ---- transpose A2T16/B2T16 -> natural ----
                A2n16 = work.tile([P, EC, K], F16, tag="A2n16")
                ps = pst.tile([P, 768], F16, tag="pst")
                for e in range(EC):
                    nc.tensor.transpose(
                        ps[:, 128 * e : 128 * (e + 1)],
                        A2T16[:, 128 * e : 128 * (e + 1)],
                        id16[:],
                    )
                nc.vector.tensor_copy(A2n16[:], ps[:, 0:E])
                B2n16 = work.tile([P, EC, K], F16, tag="B2n16")
                ps = pst.tile([P, 768], F16, tag="pst")
                for e in range(EC):
                    nc.tensor.transpose(
                        ps[:, 128 * e : 128 * (e + 1)],
                        B2T16[:, 128 * e : 128 * (e + 1)],
                        id16[:],
                    )
                nc.vector.tensor_copy(B2n16[:], ps[:, 0:E])

                # ---- S7: wqqcT psum [K=P, LV]; h_vT = tanh(wv_vT + wqqcT) ----
                hv16 = work.tile([P, LV], F16, tag="hv16")
                hvT16 = work.tile([P, LV], F16, tag="hvT16", bufs=2)
                for lo, hi in ((0, 288), (288, 576)):
                    ps = psm.tile([P, 512], F32, tag="psm")
                    for e in range(EC):
                        nc.tensor.matmul(
                            ps[:, 0 : hi - lo],
                            A2n16[:, e, :],
                            Itr16[:, e, lo:hi],
                            start=(e == 0),
                            stop=(e == EC - 1),
                        )
                    nc.vector.tensor_add(hv16[:, lo:hi], ps[:, 0 : hi - lo], wvvT16[:, lo:hi])
                nc.scalar.activation(hvT16[:], hv16[:], TANH)

                # ---- S8: wvvcT psum [K=P, LT]; h_qT = tanh(wq_qT + wvvcT) ----
                hq16 = work.tile([P, LT], F16, tag="hq16")
                hqT16 = work.tile([P, LT], F16, tag="hqT16", bufs=2)
                for h in range(2):
                    ps = psm.tile([P, 512], F32, tag="psm")
                    for e in range(EC):
                        nc.tensor.matmul(
                            ps[:],
                            B2n16[:, e, :],
                            Ttr16[:, e, 512 * h : 512 * (h + 1)],
                            start=(e == 0),
                            stop=(e == EC - 1),
                        )
                    nc.vector.tensor_add(
                        hq16[:, 512 * h : 512 * (h + 1)], ps[:],
                        wqqT16[:, 512 * h : 512 * (h + 1)],
                    )
                nc.scalar.activation(hqT16[:], hq16[:], TANH)

                return Tn16, In16, hvT16, hqT16

            def emit_tail(b, Tn16, In16, hvT16, hqT16):
                """logits -> softmax -> contexts -> Scol column for batch b."""

                def softmax_row(hT16, L, t_pre, w_sb_hx):
                    l32 = work.tile([1, L], F32, tag=t_pre + "_l")
                    for lo, hi in ((0, 512), (512, L)) if L > 512 else ((0, L),):
                        ps = pss.tile([1, 512], F32, tag="pss")
                        nc.tensor.matmul(
                            ps[0:1, 0 : hi - lo], w_sb_hx[:], hT16[:, lo:hi],
                            start=True, stop=True,
                        )
                        nc.vector.tensor_copy(l32[:, lo:hi], ps[0:1, 0 : hi - lo])
                    m32 = work.tile([1, 1], F32, tag=t_pre + "_m")
                    nc.vector.reduce_max(
                        m32[:], l32[:], axis=mybir.AxisListType.X, negate=True
                    )
                    e32 = work.tile([1, L], F32, tag=t_pre + "_e")
                    nc.scalar.activation(e32[:], l32[:], EXP, bias=m32[:])
                    s32 = work.tile([1, 1], F32, tag=t_pre + "_s")
                    nc.vector.reduce_sum(s32[:], e32[:], axis=mybir.AxisListType.X)
                    r32 = work.tile([1, 1], F32, tag=t_pre + "_r")
                    nc.vector.reciprocal(r32[:], s32[:])
                    a32 = work.tile([1, L], F32, tag=t_pre + "_a")
                    nc.vector.tensor_scalar_mul(a32[:], e32[:], r32[:])
                    return a32

                av32 = softmax_row(hvT16, LV, "av", whv_sb)
                aq32 = softmax_row(hqT16, LT, "aq", whq_sb)

                # ---- transpose av/aq into column vectors (f16) ----
                avT16 = work.tile([P, LVC], F16, tag="avT16")
                ps = pstt.tile([P, 512], F32, tag="pstt")
                for cy in range(LVC):
                    pc = LV_CH[cy]
                    nc.tensor.transpose(
                        ps[0:pc, cy : cy + 1],
                        av32[0:1, 128 * cy : 128 * cy + pc],
                        id32[0:1, 0:1],
                    )
                nc.vector.tensor_copy(avT16[:, 0:4], ps[:, 0:4])
                nc.vector.tensor_copy(avT16[0:64, 4:5], ps[0:64, 4:5])
                aqT16 = work.tile([P, LTC], F16, tag="aqT16")
                ps = pstt.tile([P, 512], F32, tag="pstt")
                for cx in range(LTC):
                    nc.tensor.transpose(
                        ps[:, cx : cx + 1],
                        aq32[0:1, 128 * cx : 128 * (cx + 1)],
                        id32[0:1, 0:1],
                    )
                nc.vector.tensor_copy(aqT16[:], ps[:, 0:LTC])

                # ---- S12: contexts, accumulated into one PSUM -> cvq32 [1, E] ----
                cvq32 = work.tile([1, E], F32, tag="cvq32")
                for h in range(2):
                    psc = pss.tile([1, 512], F32, tag="pss")
                    for cy in range(LVC):
                        pc = LV_CH[cy]
                        nc.tensor.matmul(
                            psc[0:1, 0:384],
                            avT16[0:pc, cy : cy + 1],
                            In16[0:pc, cy, 384 * h : 384 * (h + 1)],
                            start=(cy == 0),
                            stop=False,
                        )
                    for cx in range(LTC):
                        nc.tensor.matmul(
                            psc[0:1, 0:384],
                            aqT16[:, cx : cx + 1],
                            Tn16[:, cx, 384 * h : 384 * (h + 1)],
                            start=False,
                            stop=(cx == LTC - 1),
                        )
                    nc.vector.tensor_copy(
                        cvq32[:, 384 * h : 384 * (h + 1)], psc[0:1, 0:384]
                    )

                # ---- scatter (cv+cq)^T into Scol16[:, :, b] ----
                ps = pstt.tile([P, 512], F32, tag="pstt")
                for e in range(EC):
                    nc.tensor.transpose(
                        ps[:, e : e + 1],
                        cvq32[0:1, 128 * e : 128 * (e + 1)],
                        id32[0:1, 0:1],
                    )
                nc.vector.tensor_copy(Scol16[:, :, b], ps[:, 0:EC])

            # ---- software-pipelined batch loop ----
            pending = None
            for b in range(BPC):
                head_tiles = emit_head(b)
                if pending is not None:
                    emit_tail(pending[0], *pending[1])
                pending = (b, head_tiles)
            emit_tail(pending[0], *pending[1])

            # ---- S13: out = tanh(S @ w_s) for all 8 batches at once ----
            for h in range(2):
                ps = psm.tile([P, 512], F32, tag="psm")
                for e in range(EC):
                    nc.tensor.matmul(
                        ps[0:BPC, 0:384],
                        Scol16[:, e, :],
                        ws_sb[:, e, 384 * h : 384 * (h + 1)],
                        start=(e == 0),
                        stop=(e == EC - 1),
                    )
                nc.scalar.activation(
                    out32[:, 384 * h : 384 * (h + 1)], ps[0:BPC, 0:384], TANH
                )
            nc.sync.dma_start(out_d[:], out32[:])

    if split_drains:
        _split_excess_waits(nc)
    return nc


_NC = None


def _get_nc():
    global _NC
    if _NC is None:
        _NC = build_nc()
    return _NC


def _make_in_maps(text, image, w_b, w_v, w_q, w_hv, w_hq, w_s):
    f16 = np.float16
    weights = {
        "wq": np.ascontiguousarray(np.asarray(w_q), dtype=f16),
        "wv": np.ascontiguousarray(np.asarray(w_v), dtype=f16),
        "wb": np.ascontiguousarray(np.asarray(w_b), dtype=f16),
        "wbT": np.ascontiguousarray(np.asarray(w_b).T, dtype=f16),
        "whv": np.ascontiguousarray(np.asarray(w_hv), dtype=f16),
        "whq": np.ascontiguousarray(np.asarray(w_hq), dtype=f16),
        "ws": np.ascontiguousarray(np.asarray(w_s), dtype=f16),
    }
    text = np.asarray(text, dtype=np.float32)
    image = np.asarray(image, dtype=np.float32)
    in_maps = []
    for c in range(N_CORES):
        sl = slice(BPC * c, BPC * (c + 1))
        in_maps.append(
            {
                "text": np.ascontiguousarray(text[sl]),
                "image": np.ascontiguousarray(image[sl]),
                **weights,
            }
        )
    return in_maps


def kernel(
    text_hidden_states,
    image_hidden_states,
    text_attention_mask,
    w_b,
    w_v,
    w_q,
    w_hv,
    w_hq,
    w_s,
    _trace=False,
):
    # text_attention_mask is all-ones and unused by the reference computation.
    in_maps = _make_in_maps(
        text_hidden_states, image_hidden_states, w_b, w_v, w_q, w_hv, w_hq, w_s
    )
    nc = _get_nc()
    res = bass_utils.run_bass_kernel_spmd(
        nc, in_maps, core_ids=list(range(N_CORES)), trace=_trace
    )
    out = np.concatenate([res.results[c]["out"] for c in range(N_CORES)], axis=0)
    if _trace:
        kernel._last_exec_time_ns = res.exec_time_ns
    return out.astype(np.float32)


kernel._last_exec_time_ns = None

